# revision 1
# baseline (speedup 1.0000x reference)
"""Trainium2 Bass kernel for CandidateRelationalActionQRanker (N=1024, F=256,
H=64, 4 heads), SPMD over 8 NeuronCores.

Sharding: pairwise i-axis (rows) across cores, 128 rows each. Input proj is
replicated; attention is row-sharded (each core's q covers its own rows, k/v
are full) and the encoded [N,H] is AllGather'd; the dense all-pairs gated
message passing runs per-core over its 128xN block; a second AllGather feeds
the global-mean score head.

Host-side weight-only restructuring:
  pf @ W = enc_i @ (Wa+Wc) + enc_j @ (Wb-Wc) + |enc_i - enc_j| @ Wd
so the N^2 matmul carries only the abs-diff term with the enc_j term folded
into the same K=128 contraction (lhsT rows = [Wd ; Wb-Wc], rhs rows =
[|d| ; enc_j^T]); the enc_i term + b1 is a per-partition bias in the relu
drain. The value head's second linear layer is hoisted out of the pair loop:
  sum_j w_ij (relu(h1_ij) @ W2 + b2) = (sum_j w_ij relu(h1_ij)) @ W2 + b2
because softmax weights sum to 1. pg_b2 is softmax-invariant and dropped.

SPMD trick: each core receives features^T with row-blocks rotated so its own
block is always columns 0:128 (attention is j-order invariant; the AllGather
restores canonical order), so all APs stay static.
"""

import sys
import types

import numpy as np

N, F, H = 1024, 256, 64
NH, HD = 4, 16
NCORES = 8
P = N // NCORES  # 128
EPS = 1e-5
NEG = -1.0e9

_CACHE = {}


# ---------------------------------------------------------------------------
# environment shims
# ---------------------------------------------------------------------------
def _install_ntff_shim():
    if "antenv.axon_hooks" in sys.modules:
        return
    try:
        import antenv
    except ImportError:
        return
    holder = {}
    mod = types.ModuleType("antenv.axon_hooks")
    mod.set_axon_ntff_profile_hook = lambda h: holder.__setitem__("h", h)
    mod.get_axon_ntff_profile_hook = lambda: holder.get("h")
    sys.modules["antenv.axon_hooks"] = mod
    antenv.axon_hooks = mod
    try:
        from trn_agent_boot.trn_boot import _ntff_profile_via_ctypes

        mod.set_axon_ntff_profile_hook(
            _ntff_profile_via_ctypes("/opt/axon/libaxon_pjrt.so")
        )
    except Exception:
        pass


def _install_drain_patch():
    """This walrus build rejects Drain instructions carrying >1 semaphore
    wait; split the TileContext kernel-tail drain waits into individual
    wait_ge instructions."""
    import concourse.tile as tile
    from concourse.vector_clock import ScopedClock

    if getattr(tile.TileContext, "_drain_patched", False):
        return

    def _patched(self, tick_clock, wait_clock):
        nc = self.nc
        nop_inst = nc.sync.nop(nofuse=True, hint="pre_drain_waits")
        wait_clock.add_sem_waits(
            nop_inst.ins, ScopedClock({None: tick_clock.global_clock})
        )
        waits = (
            list(nop_inst.ins.sync_info.on_wait or [])
            if nop_inst.ins.sync_info
            else []
        )
        if nop_inst.ins.sync_info:
            nop_inst.ins.sync_info.on_wait = waits[:1]
        handles = {h.name: h for h in self.sems.allocated().values()}
        for w in waits[1:]:
            h = handles.get(w.ant_name)
            assert h is not None, f"no semaphore handle for {w.ant_name}"
            assert w.wait_mode == "sem-ge-imm", w.wait_mode
            nc.sync.wait_ge(h, w.wait_value)
        nc.sync.drain()
        nc.all_engine_barrier()
        popped = nc._tile_sem_poison_stack.pop()
        assert popped is self._sem_poison
        nc.clear_and_free_semaphores(list(self.sems.allocated().values()))
        nc.all_engine_barrier()

    tile.TileContext._drain_and_barrier = _patched
    tile.TileContext._drain_patched = True


# ---------------------------------------------------------------------------
# host-side preprocessing (weights only + pure input marshaling)
# ---------------------------------------------------------------------------
def _host_prep(inp):
    import ml_dtypes

    bf16 = ml_dtypes.bfloat16
    F32 = np.float32
    f = lambda x: np.ascontiguousarray(np.asarray(x, F32))

    featT = f(inp["features"]).T  # [256, 1024] (view; rolled per core below)

    w1 = f(inp["ip_w1"])
    b1 = f(inp["ip_b1"])
    win = f(inp["attn_in_w"])
    binn = f(inp["attn_in_b"])
    wq, wk, wv = win[:, 0:H], win[:, H : 2 * H], win[:, 2 * H :]
    bq, bk, bv = binn[0:H], binn[H : 2 * H], binn[2 * H :]
    s = 1.0 / np.sqrt(np.sqrt(HD))

    def aug(w, b):
        return np.concatenate([w, np.asarray(b, F32).reshape(1, -1)], 0)

    def split_pair(w):
        wa, wb, wc, wd = w[0:H], w[H : 2 * H], w[2 * H : 3 * H], w[3 * H :]
        return wa + wc, wb - wc, wd

    Av, Bv, Wdv = split_pair(f(inp["pv_w1"]))
    Ag, Bg, Wdg = split_pair(f(inp["pg_w1"]))
    # |d| = 2*relu(e) - e with e = enc_j - enc_i, so the abs folds into the
    # weights: Wd^T|d| = (2Wd)^T relu(e) + (-Wd)^T enc_j + Wd^T enc_i.
    Wbig = np.zeros((128, 128), F32)
    Wbig[0:64, 0:64] = 2.0 * Wdg
    Wbig[0:64, 64:128] = 2.0 * Wdv
    Wbig[64:128, 0:64] = Bg - Wdg
    Wbig[64:128, 64:128] = Bv - Wdv
    Astack = np.zeros((65, 128), F32)
    Astack[0:64, 0:64] = Ag + Wdg
    Astack[0:64, 64:128] = Av + Wdv
    Astack[64, 0:64] = f(inp["pg_b1"])
    Astack[64, 64:128] = f(inp["pv_b1"])

    sh_w1 = f(inp["sh_w1"])
    t128 = lambda v: np.ascontiguousarray(
        np.broadcast_to(f(v).reshape(1, -1), (128, 64))
    )

    shared = {
        "w1a": f(w1[0:128]),
        "w1b": f(w1[128:256]),
        "b1r": f(b1).reshape(1, 64),
        "w2aug": aug(f(inp["ip_w2"]), inp["ip_b2"]),
        "ipg": t128(inp["ip_ln_g"]),
        "ipb": t128(inp["ip_ln_b"]),
        "wqa": aug(wq * s, bq * s),
        "wka": aug(wk * s, bk * s),
        "wva": aug(wv, bv),
        "woa": aug(f(inp["attn_out_w"]), inp["attn_out_b"]).astype(bf16),
        "atg": t128(inp["attn_ln_g"]),
        "atb": t128(inp["attn_ln_b"]),
        "Wbig": Wbig,
        "Astack": Astack,
        "w2vec": f(inp["pg_w2"]).reshape(64, 1).astype(bf16),
        "pvw2a": aug(f(inp["pv_w2"]), inp["pv_b2"]),
        "ctg": t128(inp["ctx_ln_g"]),
        "ctb": t128(inp["ctx_ln_b"]),
        "shW1a": f(sh_w1[0:H]),
        "shW1b": f(sh_w1[H:]),
        "sh_b1": f(inp["sh_b1"]).reshape(64, 1),
        "shw2": f(inp["sh_w2"]).reshape(64, 1).astype(bf16),
        "idf": np.eye(128, dtype=F32),
        "idb": np.eye(128, dtype=bf16),
        "ones1": np.ones((1, 128), F32),
    }
    sh_b2 = float(np.asarray(inp["sh_b2"], F32).reshape(-1)[0])

    per_core = []
    for c in range(NCORES):
        perm = np.r_[c * P : N, 0 : c * P]
        mask = np.zeros((P, N), F32)
        mask[np.arange(P), c * P + np.arange(P)] = NEG
        m = dict(shared)
        m["featT0"] = np.ascontiguousarray(featT[0:128][:, perm])
        m["featT1"] = np.ascontiguousarray(featT[128:256][:, perm])
        m["diagmask"] = mask
        per_core.append(m)
    return per_core, sh_b2


# ---------------------------------------------------------------------------
# device program
# ---------------------------------------------------------------------------
_IN_SPECS = [
    ("featT0", [128, 1024], "f32"),
    ("featT1", [128, 1024], "f32"),
    ("w1a", [128, 64], "f32"),
    ("w1b", [128, 64], "f32"),
    ("b1r", [1, 64], "f32"),
    ("w2aug", [65, 64], "f32"),
    ("ipg", [128, 64], "f32"),
    ("ipb", [128, 64], "f32"),
    ("wqa", [65, 64], "f32"),
    ("wka", [65, 64], "f32"),
    ("wva", [65, 64], "f32"),
    ("woa", [65, 64], "bf16"),
    ("atg", [128, 64], "f32"),
    ("atb", [128, 64], "f32"),
    ("Wbig", [128, 128], "f32"),
    ("Astack", [65, 128], "f32"),
    ("w2vec", [64, 1], "bf16"),
    ("pvw2a", [65, 64], "f32"),
    ("ctg", [128, 64], "f32"),
    ("ctb", [128, 64], "f32"),
    ("shW1a", [64, 64], "f32"),
    ("shW1b", [64, 64], "f32"),
    ("sh_b1", [64, 1], "f32"),
    ("shw2", [64, 1], "bf16"),
    ("idf", [128, 128], "f32"),
    ("idb", [128, 128], "bf16"),
    ("ones1", [1, 128], "f32"),
    ("diagmask", [128, 1024], "f32"),
]



def _split_multi_waits(nc, mybir, template_ins, limit=1):
    """This walrus build caps sync waits per instruction at 1; move extra
    waits onto standalone nops inserted just before the instruction (same
    engine, so program order preserves semantics)."""
    import copy

    counter = [0]

    def mknop(engine, wait):
        counter[0] += 1
        nop = copy.deepcopy(template_ins)
        nop.name = f"WSPL-{counter[0]}"
        nop.engine = engine
        nop.sync_info = mybir.SyncInfo(on_wait=[wait], on_update=[])
        return nop

    for bb in nc.main_func.blocks:
        out = []
        changed = False
        for ins in bb.instructions:
            si = ins.sync_info
            if si is not None and si.on_wait and len(si.on_wait) > limit:
                waits = list(si.on_wait)
                for w in waits[:-limit]:
                    out.append(mknop(ins.engine, w))
                si.on_wait = waits[-limit:]
                ins.sync_info = si
                changed = True
            out.append(ins)
        if changed:
            bb.instructions = out


def _build(sh_b2):
    import concourse.bass as bass
    import concourse.mybir as mybir
    import concourse.tile as tile

    _install_drain_patch()
    dt = mybir.dt
    AL = mybir.AluOpType
    AF = mybir.ActivationFunctionType
    DT = {"f32": dt.float32, "bf16": dt.bfloat16}

    nc = bass.Bass("TRN2", target_bir_lowering=False, debug=False, num_devices=NCORES)
    dram_in = {
        name: nc.dram_tensor(name, shape, DT[d], kind="ExternalInput")
        for name, shape, d in _IN_SPECS
    }
    out_dram = nc.dram_tensor("out", [1, 128], dt.float32, kind="ExternalOutput")
    RG = [list(range(NCORES))]

    with tile.TileContext(nc) as tc:
        _tmpl = nc.sync.nop(nofuse=True, hint="wait_split_template")
        with (
            tc.tile_pool(name="const", bufs=1) as cpool,
            tc.tile_pool(name="persist", bufs=1) as ppool,
            tc.tile_pool(name="dram", bufs=1, space="DRAM") as dpool,
        ):
            S1_CONSTS = {"featT0", "featT1", "w1a", "w1b", "b1r", "w2aug",
                         "ipg", "ipb"}
            c_ = {}
            for name, shape, d in _IN_SPECS:
                if name in S1_CONSTS:
                    continue
                tl = cpool.tile(shape, DT[d], tag=f"c_{name}", name=f"c_{name}")
                nc.sync.dma_start(tl[:], dram_in[name][:])
                c_[name] = tl

            # persistent activations
            hT = ppool.tile([65, 1024], dt.float32, tag="hT")
            nc.vector.memset(hT[64:65, :], 1.0)
            h_rows = ppool.tile([128, 512], dt.float32, tag="h_rows")
            kTh = [
                ppool.tile([16, 1024], dt.bfloat16, tag=f"kTh{h}", name=f"kTh{h}")
                for h in range(NH)
            ]
            qTh = [
                ppool.tile([16, 128], dt.bfloat16, tag=f"qTh{h}", name=f"qTh{h}")
                for h in range(NH)
            ]
            vt = ppool.tile([128, 512], dt.bfloat16, tag="vt")
            oS = ppool.tile([128, 64], dt.bfloat16, tag="oS")
            oSTa = ppool.tile([65, 128], dt.bfloat16, tag="oSTa")
            nc.vector.memset(oSTa[64:65, :], 1.0)
            encO = ppool.tile([128, 64], dt.float32, tag="encO")
            encTO = ppool.tile([65, 128], dt.float32, tag="encTO")
            nc.vector.memset(encTO[64:65, :], 1.0)
            encT = ppool.tile([64, 1024], dt.float32, tag="encT")
            T_all = ppool.tile([128, 128], dt.float32, tag="T_all")
            r_aug = ppool.tile([65, 128], dt.float32, tag="r_aug")
            nc.vector.memset(r_aug[64:65, :], 1.0)
            Wsoft = ppool.tile([128, 1024], dt.bfloat16, tag="Wsoft")
            enc2 = ppool.tile([128, 64], dt.float32, tag="enc2")
            onescol = ppool.tile([128, 1], dt.float32, tag="onescol")
            nc.vector.memset(onescol[:], 1.0)
            epsc = ppool.tile([128, 1], dt.float32, tag="epsc")
            nc.vector.memset(epsc[:], EPS)
            Rt = [
                ppool.tile([128, 1024], dt.float32, tag=f"R{k}", name=f"R{k}")
                for k in range(4)
            ]

            def layernorm(sp, psum_src, gt, bt, out_ap, relu, tag):
                s_ = sp.tile([128, 1], dt.float32, tag=f"s_{tag}")
                nc.vector.tensor_reduce(
                    out=s_[:], in_=psum_src, axis=mybir.AxisListType.X, op=AL.add
                )
                nm = sp.tile([128, 1], dt.float32, tag=f"nm_{tag}")
                nc.vector.tensor_scalar(
                    out=nm[:], in0=s_[:], scalar1=-1.0 / 64.0, scalar2=None,
                    op0=AL.mult,
                )
                zc = sp.tile([128, 64], dt.float32, tag=f"zc_{tag}")
                nc.vector.tensor_scalar(
                    out=zc[:], in0=psum_src, scalar1=nm[:], scalar2=None, op0=AL.add
                )
                sq = sp.tile([128, 64], dt.float32, tag=f"sq_{tag}")
                ss = sp.tile([128, 1], dt.float32, tag=f"ss_{tag}")
                nc.scalar.activation(
                    out=sq[:], in_=zc[:], func=AF.Square, accum_out=ss[:]
                )
                sr = sp.tile([128, 1], dt.float32, tag=f"sr_{tag}")
                nc.scalar.activation(
                    out=sr[:], in_=ss[:], func=AF.Sqrt, bias=epsc[0 : ss.shape[0], :],
                    scale=1.0 / 64.0,
                )
                rstd = sp.tile([128, 1], dt.float32, tag=f"rstd_{tag}")
                nc.vector.reciprocal(out=rstd[:], in_=sr[:])
                xh = sp.tile([128, 64], dt.float32, tag=f"xh_{tag}")
                nc.vector.tensor_scalar(
                    out=xh[:], in0=zc[:], scalar1=rstd[:], scalar2=None, op0=AL.mult
                )
                xg = sp.tile([128, 64], dt.float32, tag=f"xg_{tag}")
                nc.vector.tensor_tensor(out=xg[:], in0=xh[:], in1=gt[:], op=AL.mult)
                if relu:
                    nc.vector.tensor_tensor(out=xg[:], in0=xg[:], in1=bt[:], op=AL.add)
                    nc.vector.tensor_scalar(
                        out=out_ap, in0=xg[:], scalar1=0.0, scalar2=None, op0=AL.max
                    )
                else:
                    nc.vector.tensor_tensor(out=out_ap, in0=xg[:], in1=bt[:], op=AL.add)

            # ================= stage 1: input proj (all rows) =============
            with (
                tc.tile_pool(name="s1p", bufs=2, space="PSUM") as s1p,
                tc.tile_pool(name="s1s", bufs=3) as s1s,
                tc.tile_pool(name="s1c", bufs=1) as s1c,
            ):
                for name, shape, d in _IN_SPECS:
                    if name not in S1_CONSTS:
                        continue
                    tl = s1c.tile(shape, DT[d], tag=f"c_{name}", name=f"c_{name}")
                    nc.sync.dma_start(tl[:], dram_in[name][:])
                    c_[name] = tl
                for rb in range(8):
                    rsl = slice(rb * 128, (rb + 1) * 128)
                    pz = s1p.tile([128, 64], dt.float32, tag="s1")
                    nc.tensor.matmul(
                        pz[:], c_["featT0"][:, rsl], c_["w1a"][:], start=True,
                        stop=False,
                    )
                    nc.tensor.matmul(
                        pz[:], c_["featT1"][:, rsl], c_["w1b"][:], start=False,
                        stop=False,
                    )
                    nc.tensor.matmul(
                        pz[:], c_["ones1"][:], c_["b1r"][:], start=False,
                        stop=True,
                    )
                    y = s1s.tile([128, 64], dt.float32, tag="y")
                    layernorm(s1s, pz[:], c_["ipg"], c_["ipb"], y[:], True, "ln1")
                    pyT = s1p.tile([64, 128], dt.float32, tag="s1")
                    nc.tensor.transpose(pyT[:], y[:], c_["idf"][:])
                    yT = s1s.tile([65, 128], dt.float32, tag="yT")
                    nc.vector.memset(yT[64:65, :], 1.0)
                    nc.vector.tensor_copy(yT[0:64, :], pyT[:])
                    ph = s1p.tile([128, 64], dt.float32, tag="s1")
                    nc.tensor.matmul(ph[:], yT[:], c_["w2aug"][:], start=True, stop=True)
                    hro = h_rows[:, rb * 64 : (rb + 1) * 64]
                    nc.vector.tensor_scalar(
                        out=hro, in0=ph[:], scalar1=0.0, scalar2=None, op0=AL.max
                    )
                    phT = s1p.tile([64, 128], dt.float32, tag="s1")
                    nc.tensor.transpose(phT[:], hro, c_["idf"][:])
                    nc.vector.tensor_copy(hT[0:64, rsl], phT[:])

                # qkv projections (per head: matmul base partitions are
                # restricted to 0/32/64, so 16-row head slices need own tiles)
                for h in range(NH):
                    hsl = slice(h * HD, (h + 1) * HD)
                    pq = s1p.tile([16, 128], dt.float32, tag="s1", name="pq")
                    nc.tensor.matmul(pq[:], c_["wqa"][:, hsl], hT[:, 0:128],
                                     start=True, stop=True)
                    nc.vector.tensor_copy(qTh[h][:], pq[:])
                    for half in range(2):
                        csl = slice(half * 512, (half + 1) * 512)
                        pk = s1p.tile([16, 512], dt.float32, tag="s1", name="pk")
                        nc.tensor.matmul(pk[:], c_["wka"][:, hsl], hT[:, csl],
                                         start=True, stop=True)
                        nc.vector.tensor_copy(kTh[h][:, csl], pk[:])
                for rb in range(8):
                    rsl = slice(rb * 128, (rb + 1) * 128)
                    pv = s1p.tile([128, 64], dt.float32, tag="s1")
                    nc.tensor.matmul(pv[:], hT[:, rsl], c_["wva"][:], start=True,
                                     stop=True)
                    nc.vector.tensor_copy(vt[:, rb * 64 : (rb + 1) * 64], pv[:])

            # ================= stage 2: attention (own 128 rows) ==========
            with (
                tc.tile_pool(name="s2ps", bufs=2, space="PSUM") as s2ps,
                tc.tile_pool(name="s2pt", bufs=3, space="PSUM") as s2pt,
                tc.tile_pool(name="s2po", bufs=1, space="PSUM") as s2po,
                tc.tile_pool(name="s2s", bufs=3) as s2s,
            ):
                po = s2po.tile([128, 64], dt.float32, tag="po")
                for h in range(NH):
                    hsl = slice(h * HD, (h + 1) * HD)
                    ps = s2ps.tile([128, 1024], dt.float32, tag="ps")
                    for half in range(2):
                        csl = slice(half * 512, (half + 1) * 512)
                        nc.tensor.matmul(
                            ps[:, csl], qTh[h][:], kTh[h][:, csl], start=True,
                            stop=True,
                        )
                    mx = s2s.tile([128, 1], dt.float32, tag="mx")
                    nc.vector.tensor_reduce(
                        out=mx[:], in_=ps[:], axis=mybir.AxisListType.X, op=AL.max
                    )
                    nmx = s2s.tile([128, 1], dt.float32, tag="nmx")
                    nc.vector.tensor_scalar(
                        out=nmx[:], in0=mx[:], scalar1=-1.0, scalar2=None, op0=AL.mult
                    )
                    Pt = s2s.tile([128, 1024], dt.bfloat16, tag="Pt")
                    rs = s2s.tile([128, 1], dt.float32, tag="rs")
                    nc.scalar.activation(
                        out=Pt[:], in_=ps[:], func=AF.Exp, bias=nmx[:], accum_out=rs[:]
                    )
                    rc = s2s.tile([128, 1], dt.float32, tag="rc")
                    nc.vector.reciprocal(out=rc[:], in_=rs[:])
                    for jb in range(8):
                        jsl = slice(jb * 128, (jb + 1) * 128)
                        pT = s2pt.tile([128, 128], dt.bfloat16, tag="tp")
                        nc.tensor.transpose(pT[:], Pt[:, jsl], c_["idb"][:])
                        PTs = s2s.tile([128, 128], dt.bfloat16, tag="PTs")
                        nc.vector.tensor_copy(PTs[:], pT[:])
                        nc.tensor.matmul(
                            po[:, hsl],
                            PTs[:],
                            vt[:, jb * 64 + h * HD : jb * 64 + (h + 1) * HD],
                            start=(jb == 0),
                            stop=(jb == 7),
                        )
                    nc.vector.tensor_scalar(
                        out=oS[:, hsl], in0=po[:, hsl], scalar1=rc[:], scalar2=None,
                        op0=AL.mult,
                    )
                # out-proj + residual + LN
                poT = s2pt.tile([64, 128], dt.bfloat16, tag="tp")
                nc.tensor.transpose(poT[:], oS[:], c_["idb"][:])
                nc.vector.tensor_copy(oSTa[0:64, :], poT[:])
                po2 = s2pt.tile([128, 64], dt.float32, tag="tp")
                nc.tensor.matmul(po2[:], oSTa[:], c_["woa"][:], start=True, stop=True)
                e0 = s2s.tile([128, 64], dt.float32, tag="e0")
                nc.vector.tensor_tensor(
                    out=e0[:], in0=po2[:], in1=h_rows[:, 0:64], op=AL.add
                )
                layernorm(s2s, e0[:], c_["atg"], c_["atb"], encO[:], False, "ln2")
                peT = s2pt.tile([64, 128], dt.float32, tag="tp")
                nc.tensor.transpose(peT[:], encO[:], c_["idf"][:])
                nc.vector.tensor_copy(encTO[0:64, :], peT[:])
                pT2 = s2pt.tile([128, 128], dt.float32, tag="tp")
                nc.tensor.matmul(pT2[:], c_["Astack"][:], encTO[:], start=True,
                                 stop=True)
                nc.vector.tensor_copy(T_all[:], pT2[:])

            # ================= stage 3: AllGather enc =====================
            agA = dpool.tile([128, 64], dt.float32, tag="agA")
            agB = dpool.tile([1024, 64], dt.float32, tag="agB")
            nc.sync.dma_start(agA[:], encO[:])
            nc.gpsimd.collective_compute(
                "AllGather",
                mybir.AluOpType.bypass,
                replica_groups=RG,
                ins=[agA.opt()],
                outs=[agB.opt()],
            )
            with (
                tc.tile_pool(name="s3p", bufs=2, space="PSUM") as s3p,
                tc.tile_pool(name="s3s", bufs=2) as s3s,
            ):
                for rb in range(8):
                    rsl = slice(rb * 128, (rb + 1) * 128)
                    ef = s3s.tile([128, 64], dt.float32, tag="ef")
                    nc.sync.dma_start(ef[:], agB[rsl, :])
                    pe = s3p.tile([64, 128], dt.float32, tag="pe")
                    nc.tensor.transpose(pe[:], ef[:], c_["idf"][:])
                    nc.vector.tensor_copy(encT[:, rsl], pe[:])
            for k in range(4):
                nc.vector.tensor_copy(Rt[k][64:128, :], encT[:])

            # ================= stage 4+5: pairwise block ==================
            with (
                tc.tile_pool(name="gtp", bufs=2, space="PSUM") as gtpsum,
                tc.tile_pool(name="gcp", bufs=1, space="PSUM") as gcpsum,
                tc.tile_pool(name="pps", bufs=2, space="PSUM") as ppsum,
                tc.tile_pool(name="h1p", bufs=64) as h1pool,
                tc.tile_pool(name="s4s", bufs=2) as s4s,
                tc.tile_pool(name="s5s", bufs=1) as s5s,
            ):
                h1_tiles = {}
                gt_tiles = {}

                def stage4_iter(i):
                    ch, ii = divmod(i, 64)
                    Rk = Rt[i % 4]
                    nc.vector.tensor_scalar(
                        out=Rk[0:64, :], in0=encT[:], scalar1=encTO[0:64, i : i + 1],
                        scalar2=0.0, op0=AL.subtract, op1=AL.max,
                    )
                    pp = ppsum.tile([128, 1024], dt.float32, tag="pp", name="pp")
                    for half in range(2):
                        csl = slice(half * 512, (half + 1) * 512)
                        nc.tensor.matmul(
                            pp[:, csl], c_["Wbig"][:], Rk[:, csl], start=True, stop=True
                        )
                    # h1 rows 0:64 = gate-h1, rows 64:128 = val-h1
                    h1 = h1pool.tile([128, 1024], dt.bfloat16, tag="h1", name="h1")
                    h1_tiles[i] = h1
                    tbias = T_all[:, i : i + 1]
                    nc.vector.tensor_scalar(
                        out=h1[:, 0:320], in0=pp[:, 0:320], scalar1=tbias,
                        scalar2=0.0, op0=AL.add, op1=AL.max,
                    )
                    nc.scalar.activation(
                        out=h1[:, 320:1024], in_=pp[:, 320:1024], func=AF.Relu,
                        bias=tbias,
                    )
                    # gate readout columns: GT[jp, cb*64+ii] = g(j=cb*128+jp, i)
                    GTp = gt_tiles[ch]
                    for cb in range(8):
                        nc.tensor.matmul(
                            GTp[:, cb * 64 + ii : cb * 64 + ii + 1],
                            h1[0:64, cb * 128 : (cb + 1) * 128],
                            c_["w2vec"][:],
                            start=True, stop=True,
                        )

                def stage5_chunk(ch):
                    rsl = slice(ch * 64, (ch + 1) * 64)
                    GTp = gt_tiles.pop(ch)
                    GTs = s5s.tile([128, 512], dt.float32, tag="GTs", name="GTs")
                    nc.vector.tensor_copy(GTs[:], GTp[:])
                    Gch = gcpsum.tile([64, 1024], dt.float32, tag="Gch", name="Gch")
                    for cb in range(8):
                        nc.tensor.transpose(
                            Gch[:, cb * 128 : (cb + 1) * 128],
                            GTs[:, cb * 64 : (cb + 1) * 64],
                            c_["idf"][:],
                        )
                    G2 = s5s.tile([64, 1024], dt.float32, tag="G2", name="G2")
                    nc.vector.tensor_tensor(
                        out=G2[:], in0=Gch[:], in1=c_["diagmask"][rsl, :], op=AL.add
                    )
                    rsum = s5s.tile([64, 1], dt.float32, tag="rsum", name="rsum")
                    nc.scalar.activation(
                        out=G2[:], in_=G2[:], func=AF.Exp, accum_out=rsum[:]
                    )
                    rcp = s5s.tile([64, 1], dt.float32, tag="rcp", name="rcp")
                    nc.vector.reciprocal(out=rcp[:], in_=rsum[:])
                    nc.vector.tensor_scalar(
                        out=Wsoft[rsl, :], in0=G2[:], scalar1=rcp[:], scalar2=None,
                        op0=AL.mult,
                    )
                    nc.sync.dma_start(Wdram[ch][:], Wsoft[rsl, :])

                def weighted_sum_iter(i):
                    ch, ii = divmod(i, 64)
                    # rows 64:128 so base partition matches h1's val half
                    wb = s4s.tile([128, 1024], dt.bfloat16, tag="wb", name="wb")
                    for half in range(2):
                        csl = slice(half * 512, (half + 1) * 512)
                        nc.sync.dma_start(
                            wb[64:128, csl],
                            Wdram[ch][ii : ii + 1, csl].to_broadcast([64, 512]),
                        )
                    scr = s4s.tile([128, 1024], dt.bfloat16, tag="scr", name="scr")
                    h1 = h1_tiles.pop(i)
                    nc.vector.scalar_tensor_tensor(
                        out=scr[64:128, :], in0=h1[64:128, :], scalar=1.0,
                        in1=wb[64:128, :], op0=AL.mult, op1=AL.mult,
                        accum_out=r_aug[0:64, i : i + 1],
                    )

                Wdram = [
                    dpool.tile([64, 1024], dt.bfloat16, tag=f"Wdram{ch}",
                               name=f"Wdram{ch}")
                    for ch in range(2)
                ]
                for ch in range(2):
                    gt_tiles[ch] = gtpsum.tile(
                        [128, 512], dt.float32, tag="GTp", name="GTp"
                    )
                    for ii in range(64):
                        stage4_iter(ch * 64 + ii)
                    stage5_chunk(ch)
                    for ii in range(64):
                        weighted_sum_iter(ch * 64 + ii)

            # ================= stage 6: ctx head + output =================
            with (
                tc.tile_pool(name="s6p", bufs=2, space="PSUM") as s6p,
                tc.tile_pool(name="s6s", bufs=2) as s6s,
            ):
                pctx = s6p.tile([128, 64], dt.float32, tag="s6")
                nc.tensor.matmul(pctx[:], r_aug[:], c_["pvw2a"][:], start=True,
                                 stop=True)
                e2 = s6s.tile([128, 64], dt.float32, tag="e2")
                nc.vector.tensor_tensor(out=e2[:], in0=pctx[:], in1=encO[:], op=AL.add)
                layernorm(s6s, e2[:], c_["ctg"], c_["ctb"], enc2[:], False, "ln3")

                agC = dpool.tile([128, 64], dt.float32, tag="agC")
                agD = dpool.tile([1024, 64], dt.float32, tag="agD")
                nc.sync.dma_start(agC[:], enc2[:])
                nc.gpsimd.collective_compute(
                    "AllGather",
                    mybir.AluOpType.bypass,
                    replica_groups=RG,
                    ins=[agC.opt()],
                    outs=[agD.opt()],
                )
                pgs = s6p.tile([64, 1], dt.float32, tag="s6")
                for rb in range(8):
                    ef2 = s6s.tile([128, 64], dt.float32, tag="ef2")
                    nc.sync.dma_start(ef2[:], agD[rb * 128 : (rb + 1) * 128, :])
                    nc.tensor.matmul(
                        pgs[:], ef2[:], onescol[:], start=(rb == 0), stop=(rb == 7)
                    )
                gsb = s6s.tile([64, 1], dt.float32, tag="gsb")
                nc.vector.tensor_scalar(
                    out=gsb[:], in0=pgs[:], scalar1=1.0 / 1024.0, scalar2=None,
                    op0=AL.mult,
                )
                pgt = s6p.tile([64, 1], dt.float32, tag="s6")
                nc.tensor.matmul(pgt[:], c_["shW1b"][:], gsb[:], start=True, stop=True)
                gbias = s6s.tile([64, 1], dt.float32, tag="gbias")
                nc.vector.tensor_tensor(
                    out=gbias[:], in0=pgt[:], in1=c_["sh_b1"][:], op=AL.add
                )
                pe2T = s6p.tile([64, 128], dt.float32, tag="s6")
                nc.tensor.transpose(pe2T[:], enc2[:], c_["idf"][:])
                e2T = s6s.tile([64, 128], dt.float32, tag="e2T")
                nc.vector.tensor_copy(e2T[:], pe2T[:])
                ps1 = s6p.tile([64, 128], dt.float32, tag="s6")
                nc.tensor.matmul(ps1[:], c_["shW1a"][:], e2T[:], start=True, stop=True)
                s1T = s6s.tile([64, 128], dt.bfloat16, tag="s1T")
                nc.scalar.activation(out=s1T[:], in_=ps1[:], func=AF.Relu, bias=gbias[:])
                pout = s6p.tile([1, 128], dt.float32, tag="s6")
                nc.tensor.matmul(pout[:], c_["shw2"][:], s1T[:], start=True, stop=True)
                outS = s6s.tile([1, 128], dt.float32, tag="outS")
                nc.vector.tensor_scalar(
                    out=outS[:], in0=pout[:], scalar1=sh_b2, scalar2=None, op0=AL.add
                )
                nc.sync.dma_start(out_dram[:], outS[:])

    _split_multi_waits(nc, mybir, _tmpl.ins)
    return nc


# ---------------------------------------------------------------------------
# runner
# ---------------------------------------------------------------------------
def _run(inputs, trace=False):
    _install_ntff_shim()
    from concourse.bass_utils import run_bass_kernel_spmd

    per_core, sh_b2 = _host_prep(inputs)
    key = "nc"
    if key not in _CACHE:
        _CACHE[key] = _build(sh_b2)
    nc = _CACHE[key]
    res = run_bass_kernel_spmd(nc, per_core, list(range(NCORES)), trace=trace)
    out = np.concatenate(
        [np.asarray(res.results[c]["out"]).reshape(-1) for c in range(NCORES)]
    ).astype(np.float32)
    return out, res


def kernel(**inputs):
    out, _ = _run(inputs, trace=False)
    return out



# revision 9
# speedup vs baseline: 1.7381x; 1.7381x over previous
"""Trainium2 Bass kernel for CandidateRelationalActionQRanker (N=1024, F=256,
H=64, 4 heads), SPMD over 8 NeuronCores.

Sharding: pairwise i-axis (rows) across cores, 128 rows each. Input proj is
replicated; attention is row-sharded (each core's q covers its own rows, k/v
are full) and the encoded [N,H] is AllGather'd; the dense all-pairs gated
message passing runs per-core over its 128xN block; a second AllGather feeds
the global-mean score head.

Host-side weight-only restructuring:
  pf @ W = enc_i @ (Wa+Wc) + enc_j @ (Wb-Wc) + |enc_i - enc_j| @ Wd
so the N^2 matmul carries only the abs-diff term with the enc_j term folded
into the same K=128 contraction (lhsT rows = [Wd ; Wb-Wc], rhs rows =
[|d| ; enc_j^T]); the enc_i term + b1 is a per-partition bias in the relu
drain. The value head's second linear layer is hoisted out of the pair loop:
  sum_j w_ij (relu(h1_ij) @ W2 + b2) = (sum_j w_ij relu(h1_ij)) @ W2 + b2
because softmax weights sum to 1. pg_b2 is softmax-invariant and dropped.

SPMD trick: each core receives features^T with row-blocks rotated so its own
block is always columns 0:128 (attention is j-order invariant; the AllGather
restores canonical order), so all APs stay static.
"""

import sys
import types

import numpy as np

N, F, H = 1024, 256, 64
NH, HD = 4, 16
NCORES = 8
P = N // NCORES  # 128
EPS = 1e-5
NEG = -1.0e9

_CACHE = {}


# ---------------------------------------------------------------------------
# environment shims
# ---------------------------------------------------------------------------
def _install_ntff_shim():
    if "antenv.axon_hooks" in sys.modules:
        return
    try:
        import antenv
    except ImportError:
        return
    holder = {}
    mod = types.ModuleType("antenv.axon_hooks")
    mod.set_axon_ntff_profile_hook = lambda h: holder.__setitem__("h", h)
    mod.get_axon_ntff_profile_hook = lambda: holder.get("h")
    sys.modules["antenv.axon_hooks"] = mod
    antenv.axon_hooks = mod
    try:
        from trn_agent_boot.trn_boot import _ntff_profile_via_ctypes

        mod.set_axon_ntff_profile_hook(
            _ntff_profile_via_ctypes("/opt/axon/libaxon_pjrt.so")
        )
    except Exception:
        pass


def _install_drain_patch():
    """This walrus build rejects Drain instructions carrying >1 semaphore
    wait; split the TileContext kernel-tail drain waits into individual
    wait_ge instructions."""
    import concourse.tile as tile
    from concourse.vector_clock import ScopedClock

    if getattr(tile.TileContext, "_drain_patched", False):
        return

    def _patched(self, tick_clock, wait_clock):
        nc = self.nc
        nop_inst = nc.sync.nop(nofuse=True, hint="pre_drain_waits")
        wait_clock.add_sem_waits(
            nop_inst.ins, ScopedClock({None: tick_clock.global_clock})
        )
        waits = (
            list(nop_inst.ins.sync_info.on_wait or [])
            if nop_inst.ins.sync_info
            else []
        )
        if nop_inst.ins.sync_info:
            nop_inst.ins.sync_info.on_wait = waits[:1]
        handles = {h.name: h for h in self.sems.allocated().values()}
        for w in waits[1:]:
            h = handles.get(w.ant_name)
            assert h is not None, f"no semaphore handle for {w.ant_name}"
            assert w.wait_mode == "sem-ge-imm", w.wait_mode
            nc.sync.wait_ge(h, w.wait_value)
        nc.sync.drain()
        nc.all_engine_barrier()
        popped = nc._tile_sem_poison_stack.pop()
        assert popped is self._sem_poison
        nc.clear_and_free_semaphores(list(self.sems.allocated().values()))
        nc.all_engine_barrier()

    tile.TileContext._drain_and_barrier = _patched
    tile.TileContext._drain_patched = True


# ---------------------------------------------------------------------------
# host-side preprocessing (weights only + pure input marshaling)
# ---------------------------------------------------------------------------
def _host_prep(inp):
    import ml_dtypes

    bf16 = ml_dtypes.bfloat16
    F32 = np.float32
    f = lambda x: np.ascontiguousarray(np.asarray(x, F32))

    featT = f(inp["features"]).T  # [256, 1024] (view; rolled per core below)

    w1 = f(inp["ip_w1"])
    b1 = f(inp["ip_b1"])
    win = f(inp["attn_in_w"])
    binn = f(inp["attn_in_b"])
    wq, wk, wv = win[:, 0:H], win[:, H : 2 * H], win[:, 2 * H :]
    bq, bk, bv = binn[0:H], binn[H : 2 * H], binn[2 * H :]
    s = 1.0 / np.sqrt(np.sqrt(HD))

    def aug(w, b):
        return np.concatenate([w, np.asarray(b, F32).reshape(1, -1)], 0)

    def split_pair(w):
        wa, wb, wc, wd = w[0:H], w[H : 2 * H], w[2 * H : 3 * H], w[3 * H :]
        return wa + wc, wb - wc, wd

    Av, Bv, Wdv = split_pair(f(inp["pv_w1"]))
    Ag, Bg, Wdg = split_pair(f(inp["pg_w1"]))
    # |d| = 2*relu(e) - e with e = enc_j - enc_i, so the abs folds into the
    # weights: Wd^T|d| = (2Wd)^T relu(e) + (-Wd)^T enc_j + Wd^T enc_i.
    # Output column order [val | gate] so the weighted-sum reads h1[0:64]
    # (partition base 0, matching the broadcast tile).
    Wbig = np.zeros((128, 128), F32)
    Wbig[0:64, 0:64] = 2.0 * Wdv
    Wbig[0:64, 64:128] = 2.0 * Wdg
    Wbig[64:128, 0:64] = Bv - Wdv
    Wbig[64:128, 64:128] = Bg - Wdg
    Astack = np.zeros((65, 128), F32)
    Astack[0:64, 0:64] = Av + Wdv
    Astack[0:64, 64:128] = Ag + Wdg
    Astack[64, 0:64] = f(inp["pv_b1"])
    Astack[64, 64:128] = f(inp["pg_b1"])

    sh_w1 = f(inp["sh_w1"])
    t128 = lambda v: np.ascontiguousarray(
        np.broadcast_to(f(v).reshape(1, -1), (128, 64))
    )

    w2vec = np.zeros((128, 1), F32)
    w2vec[64:128, 0] = f(inp["pg_w2"]).reshape(-1)

    shared = {
        "w1a": f(w1[0:128]),
        "w1b": f(w1[128:256]),
        "b1r": f(b1).reshape(1, 64),
        "w2aug": aug(f(inp["ip_w2"]), inp["ip_b2"]),
        "ipg": t128(inp["ip_ln_g"]),
        "ipb": t128(inp["ip_ln_b"]),
        "wqa": aug(wq * s, bq * s),
        "wka": aug(wk * s, bk * s),
        "wva": aug(wv, bv),
        "woa": aug(f(inp["attn_out_w"]), inp["attn_out_b"]).astype(bf16),
        "atg": t128(inp["attn_ln_g"]),
        "atb": t128(inp["attn_ln_b"]),
        "Wbig": Wbig.astype(bf16),
        "Astack": Astack,
        "w2vec": w2vec.astype(bf16),
        "pvw2a": aug(f(inp["pv_w2"]), inp["pv_b2"]),
        "ctg": t128(inp["ctx_ln_g"]),
        "ctb": t128(inp["ctx_ln_b"]),
        "shW1a": f(sh_w1[0:H]),
        "shW1b": f(sh_w1[H:]),
        "sh_b1": f(inp["sh_b1"]).reshape(64, 1),
        "shw2": f(inp["sh_w2"]).reshape(64, 1).astype(bf16),
        "idf": np.eye(128, dtype=F32),
        "idb": np.eye(128, dtype=bf16),
        "ones1": np.ones((1, 128), F32),
    }
    sh_b2 = float(np.asarray(inp["sh_b2"], F32).reshape(-1)[0])

    per_core = []
    for c in range(NCORES):
        perm = np.r_[c * P : N, 0 : c * P]
        # mask in GT layout: for chunk cc the GT column block is
        # [cc*128, cc*128+128) with col = cb*16 + ii (cb = j-block, ii =
        # i-in-chunk); the diagonal j for i = cc*16+ii sits in j-block
        # cb = core at row jp = cc*16+ii.
        mgt = np.zeros((P, N), F32)
        for cc in range(8):
            for ii in range(16):
                mgt[cc * 16 + ii, cc * 128 + c * 16 + ii] = NEG
        m = dict(shared)
        m["featT0"] = np.ascontiguousarray(featT[0:128][:, perm])
        m["featT1"] = np.ascontiguousarray(featT[128:256][:, perm])
        m["maskGT"] = mgt
        per_core.append(m)
    return per_core, sh_b2


# ---------------------------------------------------------------------------
# device program
# ---------------------------------------------------------------------------
_IN_SPECS = [
    ("featT0", [128, 1024], "f32"),
    ("featT1", [128, 1024], "f32"),
    ("w1a", [128, 64], "f32"),
    ("w1b", [128, 64], "f32"),
    ("b1r", [1, 64], "f32"),
    ("w2aug", [65, 64], "f32"),
    ("ipg", [128, 64], "f32"),
    ("ipb", [128, 64], "f32"),
    ("wqa", [65, 64], "f32"),
    ("wka", [65, 64], "f32"),
    ("wva", [65, 64], "f32"),
    ("woa", [65, 64], "bf16"),
    ("atg", [128, 64], "f32"),
    ("atb", [128, 64], "f32"),
    ("Wbig", [128, 128], "bf16"),
    ("Astack", [65, 128], "f32"),
    ("w2vec", [128, 1], "bf16"),
    ("pvw2a", [65, 64], "f32"),
    ("ctg", [128, 64], "f32"),
    ("ctb", [128, 64], "f32"),
    ("shW1a", [64, 64], "f32"),
    ("shW1b", [64, 64], "f32"),
    ("sh_b1", [64, 1], "f32"),
    ("shw2", [64, 1], "bf16"),
    ("idf", [128, 128], "f32"),
    ("idb", [128, 128], "bf16"),
    ("ones1", [1, 128], "f32"),
    ("maskGT", [128, 1024], "f32"),
]



def _split_multi_waits(nc, mybir, template_ins, limit=1):
    """This walrus build caps sync waits per instruction at 1; move extra
    waits onto standalone nops inserted just before the instruction (same
    engine, so program order preserves semantics)."""
    import copy

    counter = [0]

    def mknop(engine, wait):
        counter[0] += 1
        nop = copy.deepcopy(template_ins)
        nop.name = f"WSPL-{counter[0]}"
        nop.engine = engine
        nop.sync_info = mybir.SyncInfo(on_wait=[wait], on_update=[])
        return nop

    for bb in nc.main_func.blocks:
        out = []
        changed = False
        for ins in bb.instructions:
            si = ins.sync_info
            if si is not None and si.on_wait and len(si.on_wait) > limit:
                waits = list(si.on_wait)
                for w in waits[:-limit]:
                    out.append(mknop(ins.engine, w))
                si.on_wait = waits[-limit:]
                ins.sync_info = si
                changed = True
            out.append(ins)
        if changed:
            bb.instructions = out


def _build(sh_b2):
    import concourse.bass as bass
    import concourse.mybir as mybir
    import concourse.tile as tile

    _install_drain_patch()
    dt = mybir.dt
    AL = mybir.AluOpType
    AF = mybir.ActivationFunctionType
    DT = {"f32": dt.float32, "bf16": dt.bfloat16}

    nc = bass.Bass("TRN2", target_bir_lowering=False, debug=False, num_devices=NCORES)
    dram_in = {
        name: nc.dram_tensor(name, shape, DT[d], kind="ExternalInput")
        for name, shape, d in _IN_SPECS
    }
    out_dram = nc.dram_tensor("out", [1, 128], dt.float32, kind="ExternalOutput")
    RG = [list(range(NCORES))]

    with tile.TileContext(nc) as tc:
        _tmpl = nc.sync.nop(nofuse=True, hint="wait_split_template")
        with (
            tc.tile_pool(name="const", bufs=1) as cpool,
            tc.tile_pool(name="persist", bufs=1) as ppool,
            tc.tile_pool(name="dram", bufs=1, space="DRAM") as dpool,
        ):
            S1_CONSTS = {"featT0", "featT1", "w1a", "w1b", "b1r", "w2aug",
                         "ipg", "ipb"}
            c_ = {}
            for name, shape, d in _IN_SPECS:
                if name in S1_CONSTS:
                    continue
                tl = cpool.tile(shape, DT[d], tag=f"c_{name}", name=f"c_{name}")
                nc.sync.dma_start(tl[:], dram_in[name][:])
                c_[name] = tl

            # persistent activations
            hT = ppool.tile([65, 1024], dt.float32, tag="hT")
            nc.vector.memset(hT[64:65, :], 1.0)
            h_rows = ppool.tile([128, 512], dt.float32, tag="h_rows")
            kTh = [
                ppool.tile([16, 1024], dt.bfloat16, tag=f"kTh{h}", name=f"kTh{h}")
                for h in range(NH)
            ]
            qTh = [
                ppool.tile([16, 128], dt.bfloat16, tag=f"qTh{h}", name=f"qTh{h}")
                for h in range(NH)
            ]
            vt = ppool.tile([128, 512], dt.bfloat16, tag="vt")
            oS = ppool.tile([128, 64], dt.bfloat16, tag="oS")
            oSTa = ppool.tile([65, 128], dt.bfloat16, tag="oSTa")
            nc.vector.memset(oSTa[64:65, :], 1.0)
            encO = ppool.tile([128, 64], dt.float32, tag="encO")
            encTO = ppool.tile([65, 128], dt.float32, tag="encTO")
            nc.vector.memset(encTO[64:65, :], 1.0)
            encTb = ppool.tile([64, 1024], dt.bfloat16, tag="encTb")
            T_all = ppool.tile([128, 128], dt.float32, tag="T_all")
            r_aug = ppool.tile([65, 128], dt.float32, tag="r_aug")
            nc.vector.memset(r_aug[64:65, :], 1.0)
            rsum_all = ppool.tile([16, 8], dt.float32, tag="rsum_all")
            rcpall = ppool.tile([128, 1], dt.float32, tag="rcpall")
            enc2 = ppool.tile([128, 64], dt.float32, tag="enc2")
            onescol = ppool.tile([128, 1], dt.float32, tag="onescol")
            nc.vector.memset(onescol[:], 1.0)
            epsc = ppool.tile([128, 1], dt.float32, tag="epsc")
            nc.vector.memset(epsc[:], EPS)
            Rt = [
                ppool.tile([128, 1024], dt.bfloat16, tag=f"R{k}", name=f"R{k}")
                for k in range(4)
            ]

            def layernorm(sp, psum_src, gt, bt, out_ap, relu, tag):
                s_ = sp.tile([128, 1], dt.float32, tag=f"s_{tag}")
                nc.vector.tensor_reduce(
                    out=s_[:], in_=psum_src, axis=mybir.AxisListType.X, op=AL.add
                )
                nm = sp.tile([128, 1], dt.float32, tag=f"nm_{tag}")
                nc.vector.tensor_scalar(
                    out=nm[:], in0=s_[:], scalar1=-1.0 / 64.0, scalar2=None,
                    op0=AL.mult,
                )
                zc = sp.tile([128, 64], dt.float32, tag=f"zc_{tag}")
                nc.vector.tensor_scalar(
                    out=zc[:], in0=psum_src, scalar1=nm[:], scalar2=None, op0=AL.add
                )
                sq = sp.tile([128, 64], dt.float32, tag=f"sq_{tag}")
                ss = sp.tile([128, 1], dt.float32, tag=f"ss_{tag}")
                nc.scalar.activation(
                    out=sq[:], in_=zc[:], func=AF.Square, accum_out=ss[:]
                )
                sr = sp.tile([128, 1], dt.float32, tag=f"sr_{tag}")
                nc.scalar.activation(
                    out=sr[:], in_=ss[:], func=AF.Sqrt, bias=epsc[0 : ss.shape[0], :],
                    scale=1.0 / 64.0,
                )
                rstd = sp.tile([128, 1], dt.float32, tag=f"rstd_{tag}")
                nc.vector.reciprocal(out=rstd[:], in_=sr[:])
                xh = sp.tile([128, 64], dt.float32, tag=f"xh_{tag}")
                nc.vector.tensor_scalar(
                    out=xh[:], in0=zc[:], scalar1=rstd[:], scalar2=None, op0=AL.mult
                )
                xg = sp.tile([128, 64], dt.float32, tag=f"xg_{tag}")
                nc.vector.tensor_tensor(out=xg[:], in0=xh[:], in1=gt[:], op=AL.mult)
                if relu:
                    nc.vector.tensor_tensor(out=xg[:], in0=xg[:], in1=bt[:], op=AL.add)
                    nc.vector.tensor_scalar(
                        out=out_ap, in0=xg[:], scalar1=0.0, scalar2=None, op0=AL.max
                    )
                else:
                    nc.vector.tensor_tensor(out=out_ap, in0=xg[:], in1=bt[:], op=AL.add)

            # ================= stage 1: input proj (all rows) =============
            with (
                tc.tile_pool(name="s1p", bufs=2, space="PSUM") as s1p,
                tc.tile_pool(name="s1s", bufs=3) as s1s,
                tc.tile_pool(name="s1c", bufs=1) as s1c,
            ):
                for name, shape, d in _IN_SPECS:
                    if name not in S1_CONSTS:
                        continue
                    tl = s1c.tile(shape, DT[d], tag=f"c_{name}", name=f"c_{name}")
                    nc.sync.dma_start(tl[:], dram_in[name][:])
                    c_[name] = tl
                for rb in range(8):
                    rsl = slice(rb * 128, (rb + 1) * 128)
                    pz = s1p.tile([128, 64], dt.float32, tag="s1")
                    nc.tensor.matmul(
                        pz[:], c_["featT0"][:, rsl], c_["w1a"][:], start=True,
                        stop=False,
                    )
                    nc.tensor.matmul(
                        pz[:], c_["featT1"][:, rsl], c_["w1b"][:], start=False,
                        stop=False,
                    )
                    nc.tensor.matmul(
                        pz[:], c_["ones1"][:], c_["b1r"][:], start=False,
                        stop=True,
                    )
                    y = s1s.tile([128, 64], dt.float32, tag="y")
                    layernorm(s1s, pz[:], c_["ipg"], c_["ipb"], y[:], True, "ln1")
                    pyT = s1p.tile([64, 128], dt.float32, tag="s1")
                    nc.tensor.transpose(pyT[:], y[:], c_["idf"][:])
                    yT = s1s.tile([65, 128], dt.float32, tag="yT")
                    nc.vector.memset(yT[64:65, :], 1.0)
                    nc.vector.tensor_copy(yT[0:64, :], pyT[:])
                    ph = s1p.tile([128, 64], dt.float32, tag="s1")
                    nc.tensor.matmul(ph[:], yT[:], c_["w2aug"][:], start=True, stop=True)
                    hro = h_rows[:, rb * 64 : (rb + 1) * 64]
                    nc.vector.tensor_scalar(
                        out=hro, in0=ph[:], scalar1=0.0, scalar2=None, op0=AL.max
                    )
                    phT = s1p.tile([64, 128], dt.float32, tag="s1")
                    nc.tensor.transpose(phT[:], hro, c_["idf"][:])
                    nc.vector.tensor_copy(hT[0:64, rsl], phT[:])

                # qkv projections (per head: matmul base partitions are
                # restricted to 0/32/64, so 16-row head slices need own tiles)
                for h in range(NH):
                    hsl = slice(h * HD, (h + 1) * HD)
                    pq = s1p.tile([16, 128], dt.float32, tag="s1", name="pq")
                    nc.tensor.matmul(pq[:], c_["wqa"][:, hsl], hT[:, 0:128],
                                     start=True, stop=True)
                    nc.vector.tensor_copy(qTh[h][:], pq[:])
                    for half in range(2):
                        csl = slice(half * 512, (half + 1) * 512)
                        pk = s1p.tile([16, 512], dt.float32, tag="s1", name="pk")
                        nc.tensor.matmul(pk[:], c_["wka"][:, hsl], hT[:, csl],
                                         start=True, stop=True)
                        nc.vector.tensor_copy(kTh[h][:, csl], pk[:])
                for rb in range(8):
                    rsl = slice(rb * 128, (rb + 1) * 128)
                    pv = s1p.tile([128, 64], dt.float32, tag="s1")
                    nc.tensor.matmul(pv[:], hT[:, rsl], c_["wva"][:], start=True,
                                     stop=True)
                    nc.vector.tensor_copy(vt[:, rb * 64 : (rb + 1) * 64], pv[:])

            # ================= stage 2: attention (own 128 rows) ==========
            with (
                tc.tile_pool(name="s2ps", bufs=2, space="PSUM") as s2ps,
                tc.tile_pool(name="s2pt", bufs=3, space="PSUM") as s2pt,
                tc.tile_pool(name="s2po", bufs=1, space="PSUM") as s2po,
                tc.tile_pool(name="s2s", bufs=3) as s2s,
            ):
                po = s2po.tile([128, 64], dt.float32, tag="po")
                for h in range(NH):
                    hsl = slice(h * HD, (h + 1) * HD)
                    ps = s2ps.tile([128, 1024], dt.float32, tag="ps")
                    for half in range(2):
                        csl = slice(half * 512, (half + 1) * 512)
                        nc.tensor.matmul(
                            ps[:, csl], qTh[h][:], kTh[h][:, csl], start=True,
                            stop=True,
                        )
                    mx = s2s.tile([128, 1], dt.float32, tag="mx")
                    nc.vector.tensor_reduce(
                        out=mx[:], in_=ps[:], axis=mybir.AxisListType.X, op=AL.max
                    )
                    nmx = s2s.tile([128, 1], dt.float32, tag="nmx")
                    nc.vector.tensor_scalar(
                        out=nmx[:], in0=mx[:], scalar1=-1.0, scalar2=None, op0=AL.mult
                    )
                    Pt = s2s.tile([128, 1024], dt.bfloat16, tag="Pt")
                    rs = s2s.tile([128, 1], dt.float32, tag="rs")
                    nc.scalar.activation(
                        out=Pt[:], in_=ps[:], func=AF.Exp, bias=nmx[:], accum_out=rs[:]
                    )
                    rc = s2s.tile([128, 1], dt.float32, tag="rc")
                    nc.vector.reciprocal(out=rc[:], in_=rs[:])
                    for jb in range(8):
                        jsl = slice(jb * 128, (jb + 1) * 128)
                        pT = s2pt.tile([128, 128], dt.bfloat16, tag="tp")
                        nc.tensor.transpose(pT[:], Pt[:, jsl], c_["idb"][:])
                        PTs = s2s.tile([128, 128], dt.bfloat16, tag="PTs")
                        nc.vector.tensor_copy(PTs[:], pT[:])
                        nc.tensor.matmul(
                            po[:, hsl],
                            PTs[:],
                            vt[:, jb * 64 + h * HD : jb * 64 + (h + 1) * HD],
                            start=(jb == 0),
                            stop=(jb == 7),
                        )
                    nc.vector.tensor_scalar(
                        out=oS[:, hsl], in0=po[:, hsl], scalar1=rc[:], scalar2=None,
                        op0=AL.mult,
                    )
                # out-proj + residual + LN
                poT = s2pt.tile([64, 128], dt.bfloat16, tag="tp")
                nc.tensor.transpose(poT[:], oS[:], c_["idb"][:])
                nc.vector.tensor_copy(oSTa[0:64, :], poT[:])
                po2 = s2pt.tile([128, 64], dt.float32, tag="tp")
                nc.tensor.matmul(po2[:], oSTa[:], c_["woa"][:], start=True, stop=True)
                e0 = s2s.tile([128, 64], dt.float32, tag="e0")
                nc.vector.tensor_tensor(
                    out=e0[:], in0=po2[:], in1=h_rows[:, 0:64], op=AL.add
                )
                layernorm(s2s, e0[:], c_["atg"], c_["atb"], encO[:], False, "ln2")
                peT = s2pt.tile([64, 128], dt.float32, tag="tp")
                nc.tensor.transpose(peT[:], encO[:], c_["idf"][:])
                nc.vector.tensor_copy(encTO[0:64, :], peT[:])
                pT2 = s2pt.tile([128, 128], dt.float32, tag="tp")
                nc.tensor.matmul(pT2[:], c_["Astack"][:], encTO[:], start=True,
                                 stop=True)
                nc.vector.tensor_copy(T_all[:], pT2[:])

            # ================= stage 3: AllGather enc =====================
            agA = dpool.tile([128, 64], dt.float32, tag="agA")
            agB = dpool.tile([1024, 64], dt.float32, tag="agB")
            nc.sync.dma_start(agA[:], encO[:])
            nc.gpsimd.collective_compute(
                "AllGather",
                mybir.AluOpType.bypass,
                replica_groups=RG,
                ins=[agA.opt()],
                outs=[agB.opt()],
            )
            with (
                tc.tile_pool(name="s3p", bufs=2, space="PSUM") as s3p,
                tc.tile_pool(name="s3s", bufs=2) as s3s,
            ):
                for rb in range(8):
                    rsl = slice(rb * 128, (rb + 1) * 128)
                    ef = s3s.tile([128, 64], dt.float32, tag="ef")
                    nc.sync.dma_start(ef[:], agB[rsl, :])
                    pe = s3p.tile([64, 128], dt.float32, tag="pe")
                    nc.tensor.transpose(pe[:], ef[:], c_["idf"][:])
                    nc.vector.tensor_copy(encTb[:, rsl], pe[:])
            for k in range(4):
                nc.vector.tensor_copy(Rt[k][64:128, :], encTb[:])

            # ================= stage 4+5: pairwise block ==================
            # 8 chunks of 16 rows; chunk cc's weighted sum runs after chunk
            # cc+1's stage4 so the broadcast DMA latency is hidden.
            CH = 16
            NCH = 8
            with (
                tc.tile_pool(name="pps", bufs=2, space="PSUM") as ppsum,
                tc.tile_pool(name="gtp", bufs=2, space="PSUM") as gtpsum,
                tc.tile_pool(name="gcp", bufs=1, space="PSUM") as gcpsum,
                tc.tile_pool(name="h1p", bufs=2 * CH) as h1pool,
                tc.tile_pool(name="wbp", bufs=2) as wbpool,
                tc.tile_pool(name="s4s", bufs=2) as s4s,
                tc.tile_pool(name="s5s", bufs=2) as s5s,
            ):
                h1_tiles = {}
                gt_tiles = {}
                wb_tiles = {}

                def stage4_iter(i):
                    ch, ii = divmod(i, CH)
                    Rk = Rt[i % 4]
                    nc.vector.tensor_scalar(
                        out=Rk[0:64, :], in0=encTb[:], scalar1=encTO[0:64, i : i + 1],
                        scalar2=0.0, op0=AL.subtract, op1=AL.max,
                    )
                    pp = ppsum.tile([128, 1024], dt.float32, tag="pp", name="pp")
                    for half in range(2):
                        csl = slice(half * 512, (half + 1) * 512)
                        nc.tensor.matmul(
                            pp[:, csl], c_["Wbig"][:], Rk[:, csl], start=True, stop=True
                        )
                    # h1 rows 0:64 = val-h1, rows 64:128 = gate-h1
                    h1 = h1pool.tile([128, 1024], dt.bfloat16, tag="h1", name="h1")
                    h1_tiles[i] = h1
                    nc.scalar.activation(
                        out=h1[:], in_=pp[:], func=AF.Relu, bias=T_all[:, i : i + 1]
                    )
                    # gate readout columns: GT[jp, cb*16+ii] = g(j=cb*128+jp, i)
                    GTp = gt_tiles[ch]
                    for cb in range(8):
                        nc.tensor.matmul(
                            GTp[:, cb * CH + ii : cb * CH + ii + 1],
                            h1[64:128, cb * 128 : (cb + 1) * 128],
                            c_["w2vec"][64:128, :],
                            start=True, stop=True,
                        )

                def stage5_chunk(ch):
                    GTp = gt_tiles.pop(ch)
                    GTs = s5s.tile([128, 128], dt.float32, tag="GTs", name="GTs")
                    nc.vector.tensor_tensor(
                        out=GTs[:], in0=GTp[:],
                        in1=c_["maskGT"][:, ch * 128 : (ch + 1) * 128], op=AL.add,
                    )
                    Gch = gcpsum.tile([CH, 1024], dt.float32, tag="Gch", name="Gch")
                    for cb in range(8):
                        nc.tensor.transpose(
                            Gch[:, cb * 128 : (cb + 1) * 128],
                            GTs[:, cb * CH : (cb + 1) * CH],
                            c_["idf"][:],
                        )
                    # unnormalized softmax: exp rows out, row-sums accumulated;
                    # the reciprocal is folded into the pair-context drain.
                    Wc = s5s.tile([CH, 1024], dt.bfloat16, tag="Wc", name="Wc")
                    nc.scalar.activation(
                        out=Wc[:], in_=Gch[:], func=AF.Exp,
                        accum_out=rsum_all[:, ch : ch + 1],
                    )
                    nc.sync.dma_start(Wdram[ch][:], Wc[:])
                    wb = wbpool.tile([64, CH * 1024], dt.bfloat16, tag="wb",
                                     name="wb")
                    wb_tiles[ch] = wb
                    nc.sync.dma_start(
                        wb[:],
                        Wdram[ch][:]
                        .rearrange("i j -> (i j)")
                        .unsqueeze(0)
                        .to_broadcast([64, CH * 1024]),
                    )

                def weighted_sum_iter(i):
                    ch, ii = divmod(i, CH)
                    wb = wb_tiles[ch]
                    scr = s4s.tile([64, 1024], dt.bfloat16, tag="scr", name="scr")
                    h1 = h1_tiles.pop(i)
                    nc.vector.scalar_tensor_tensor(
                        out=scr[:], in0=h1[0:64, :], scalar=1.0,
                        in1=wb[:, ii * 1024 : (ii + 1) * 1024],
                        op0=AL.mult, op1=AL.mult,
                        accum_out=r_aug[0:64, i : i + 1],
                    )
                    if ii == CH - 1:
                        wb_tiles.pop(ch)

                Wdram = [
                    dpool.tile([CH, 1024], dt.bfloat16, tag=f"Wdram{ch}",
                               name=f"Wdram{ch}")
                    for ch in range(NCH)
                ]
                for ch in range(NCH):
                    gt_tiles[ch] = gtpsum.tile(
                        [128, 128], dt.float32, tag="GTp", name="GTp"
                    )
                    for ii in range(CH):
                        stage4_iter(ch * CH + ii)
                    stage5_chunk(ch)
                    if ch > 0:
                        for ii in range(CH):
                            weighted_sum_iter((ch - 1) * CH + ii)
                for ii in range(CH):
                    weighted_sum_iter((NCH - 1) * CH + ii)
                # per-row softmax denominators -> [128,1] column
                rcps = s5s.tile([16, 8], dt.float32, tag="rcps", name="rcps")
                nc.vector.reciprocal(out=rcps[:], in_=rsum_all[:])
                for ch in range(NCH):
                    nc.sync.dma_start(
                        rcpall[ch * CH : (ch + 1) * CH, :], rcps[:, ch : ch + 1]
                    )

            # ================= stage 6: ctx head + output =================
            with (
                tc.tile_pool(name="s6p", bufs=4, space="PSUM") as s6p,
                tc.tile_pool(name="s6s", bufs=2) as s6s,
            ):
                pctx = s6p.tile([128, 64], dt.float32, tag="s6")
                nc.tensor.matmul(pctx[:], r_aug[:], c_["pvw2a"][:], start=True,
                                 stop=True)
                # e2 = pctx * (1/rowsum) + encO  (softmax normalization folded)
                e2 = s6s.tile([128, 64], dt.float32, tag="e2")
                nc.vector.scalar_tensor_tensor(
                    out=e2[:], in0=pctx[:], scalar=rcpall[:], in1=encO[:],
                    op0=AL.mult, op1=AL.add,
                )
                layernorm(s6s, e2[:], c_["ctg"], c_["ctb"], enc2[:], False, "ln3")

                # own-block column sum -> [1,64] row, AllGather partials,
                # reduce on-core; head matmul prep runs before the collective.
                pcs = s6p.tile([64, 1], dt.float32, tag="s6")
                nc.tensor.matmul(pcs[:], enc2[:], onescol[:], start=True, stop=True)
                cs = s6s.tile([64, 1], dt.float32, tag="cs")
                nc.vector.tensor_copy(cs[:], pcs[:])
                pcsT = s6p.tile([1, 64], dt.float32, tag="s6")
                nc.tensor.transpose(pcsT[:], cs[:], c_["idf"][0:64, 0:64])
                csr = s6s.tile([1, 64], dt.float32, tag="csr")
                nc.vector.tensor_copy(csr[:], pcsT[:])
                agC = dpool.tile([1, 64], dt.float32, tag="agC")
                agD = dpool.tile([8, 64], dt.float32, tag="agD")
                nc.sync.dma_start(agC[:], csr[:])
                nc.gpsimd.collective_compute(
                    "AllGather",
                    mybir.AluOpType.bypass,
                    replica_groups=RG,
                    ins=[agC.opt()],
                    outs=[agD.opt()],
                )
                pe2T = s6p.tile([64, 128], dt.float32, tag="s6")
                nc.tensor.transpose(pe2T[:], enc2[:], c_["idf"][:])
                e2T = s6s.tile([64, 128], dt.float32, tag="e2T")
                nc.vector.tensor_copy(e2T[:], pe2T[:])
                ps1 = s6p.tile([64, 128], dt.float32, tag="s6")
                nc.tensor.matmul(ps1[:], c_["shW1a"][:], e2T[:], start=True, stop=True)

                ag8 = s6s.tile([8, 64], dt.float32, tag="ag8")
                nc.sync.dma_start(ag8[:], agD[:])
                pgs = s6p.tile([64, 1], dt.float32, tag="s6")
                nc.tensor.matmul(pgs[:], ag8[:], onescol[0:8, :], start=True,
                                 stop=True)
                gsb = s6s.tile([64, 1], dt.float32, tag="gsb")
                nc.vector.tensor_scalar(
                    out=gsb[:], in0=pgs[:], scalar1=1.0 / 1024.0, scalar2=None,
                    op0=AL.mult,
                )
                pgt = s6p.tile([64, 1], dt.float32, tag="s6")
                nc.tensor.matmul(pgt[:], c_["shW1b"][:], gsb[:], start=True, stop=True)
                gbias = s6s.tile([64, 1], dt.float32, tag="gbias")
                nc.vector.tensor_tensor(
                    out=gbias[:], in0=pgt[:], in1=c_["sh_b1"][:], op=AL.add
                )
                s1T = s6s.tile([64, 128], dt.bfloat16, tag="s1T")
                nc.scalar.activation(out=s1T[:], in_=ps1[:], func=AF.Relu, bias=gbias[:])
                pout = s6p.tile([1, 128], dt.float32, tag="s6")
                nc.tensor.matmul(pout[:], c_["shw2"][:], s1T[:], start=True, stop=True)
                outS = s6s.tile([1, 128], dt.float32, tag="outS")
                nc.vector.tensor_scalar(
                    out=outS[:], in0=pout[:], scalar1=sh_b2, scalar2=None, op0=AL.add
                )
                nc.sync.dma_start(out_dram[:], outS[:])

    _split_multi_waits(nc, mybir, _tmpl.ins)
    return nc


# ---------------------------------------------------------------------------
# runner
# ---------------------------------------------------------------------------
def _run(inputs, trace=False):
    _install_ntff_shim()
    from concourse.bass_utils import run_bass_kernel_spmd

    per_core, sh_b2 = _host_prep(inputs)
    key = "nc"
    if key not in _CACHE:
        _CACHE[key] = _build(sh_b2)
    nc = _CACHE[key]
    res = run_bass_kernel_spmd(nc, per_core, list(range(NCORES)), trace=trace)
    out = np.concatenate(
        [np.asarray(res.results[c]["out"]).reshape(-1) for c in range(NCORES)]
    ).astype(np.float32)
    return out, res


def kernel(**inputs):
    out, _ = _run(inputs, trace=False)
    return out

